# revision 6
# baseline (speedup 1.0000x reference)
"""Chamfer-distance (squared-L2, PyTorch3D defaults) kernel for 8 Trainium2 NeuronCores.

Problem (hardcoded): B=4, N=M=8192, D=3, fp32 inputs, scalar fp32 output:
    loss = 0.01 * (mean_n min_m d2[b,n,m] + mean_m min_n d2[b,n,m])
    d2[b,n,m] = |x[b,n] - y[b,m]|^2

Sharding: 8 cores = (4 batches) x (2 halves of the source points N).
Each core computes, for its (batch b, half h):
  orientation A: s[n,m] = y2[m] - 2*x.y   for n in its 4096-half, all m  -> row mins
  orientation B: t[m,n] = x2[n] - 2*y.x   for all m, n in its half      -> col (half) mins
The fixed per-row terms (x2[n] for A, y2[m] for B) are added on the host after
the min (exact: min_m(x2+s) = x2 + min_m s).

The pairwise terms are evaluated on the TensorEngine as a K=21 matmul using a
bf16 triple-split of every fp32 operand (hi/mid/lo), keeping the 6 largest
cross products. bf16xbf16 products are exact in fp32 accumulation, so this is
fp32-quality (measured rel err ~3e-6 on the final loss) at full bf16 PE speed
(fp32 matmul would stream 4x slower). Extra K rows are free: the PE streams
one moving column per cycle for any K<=128.

The VectorEngine min-reduces the PSUM tiles ([128, 2048] = 4 banks per
instruction, double/triple-buffered against the PE). The host performs the
final tiny min/mean combine in float64 (exact).
"""

import numpy as np
import ml_dtypes

B, N, M, D = 4, 8192, 8192, 3
NCORES = 8
HALF = N // 2          # 4096 source points per core
K = 21                 # 3 rows for y2-split + 3 dims * 6 product terms
# (i, j) index pairs into (hi, mid, lo) splits of the two factors; terms with
# magnitude <= 2^-24 relative (mid*lo, lo*mid, lo*lo, ...) are dropped.
TERMS = [(0, 0), (0, 1), (1, 0), (0, 2), (2, 0), (1, 1)]

_BF16 = ml_dtypes.bfloat16
_PROGRAM_CACHE = {}


def _split3(v32):
    """fp32 vector -> (hi, mid, lo) bf16 parts with hi+mid+lo ~= v (rel err ~2^-27)."""
    hi = v32.astype(_BF16)
    r = v32 - hi.astype(np.float32)
    mid = r.astype(_BF16)
    r2 = r - mid.astype(np.float32)
    lo = r2.astype(_BF16)
    return (hi, mid, lo)


def _build_rows(xside, yside, y2):
    """Rows (lhs [K, n] bf16, rhs [K, m] bf16) such that
    (lhs.T @ rhs)[n, m] ~= y2[m] - 2 * xside[n] . yside[m]  (fp32 accumulation)."""
    n = xside.shape[0]
    lhs_rows, rhs_rows = [], []
    ones = np.ones(n, dtype=_BF16)
    for part in _split3(y2):
        lhs_rows.append(ones)
        rhs_rows.append(part)
    for d in range(D):
        a = _split3(np.ascontiguousarray(xside[:, d]))
        b = _split3(np.ascontiguousarray(-2.0 * yside[:, d]))
        for (i, j) in TERMS:
            lhs_rows.append(a[i])
            rhs_rows.append(b[j])
    return np.ascontiguousarray(np.stack(lhs_rows)), np.ascontiguousarray(
        np.stack(rhs_rows)
    )


def _build_program():
    """Build + compile the (SPMD, identical on all 8 cores) Bass program."""
    import concourse.mybir as mybir
    import concourse.tile as tile
    from concourse import bacc

    dt = mybir.dt
    nc = bacc.Bacc(
        "TRN2",
        target_bir_lowering=False,
        debug=False,
        enable_asserts=False,
        num_devices=NCORES,
    )

    lhsA = nc.dram_tensor("lhsA", [K, HALF], dt.bfloat16, kind="ExternalInput").ap()
    rhsA = nc.dram_tensor("rhsA", [K, M], dt.bfloat16, kind="ExternalInput").ap()
    lhsB = nc.dram_tensor("lhsB", [K, M], dt.bfloat16, kind="ExternalInput").ap()
    rhsB = nc.dram_tensor("rhsB", [K, HALF], dt.bfloat16, kind="ExternalInput").ap()
    minsA = nc.dram_tensor(
        "minsA", [128, HALF // 128], dt.float32, kind="ExternalOutput"
    ).ap()
    minsB = nc.dram_tensor(
        "minsB", [128, M // 128], dt.float32, kind="ExternalOutput"
    ).ap()

    with tile.TileContext(nc) as tc:
        with (
            tc.tile_pool(name="inp", bufs=1) as inpool,
            tc.tile_pool(name="psum", bufs=2, space="PSUM") as pspool,
            tc.tile_pool(name="wm", bufs=2) as wmpool,
            tc.tile_pool(name="outp", bufs=1) as outpool,
        ):
            lhsA_sb = inpool.tile([K, HALF], dt.bfloat16, tag="lhsA")
            nc.sync.dma_start(lhsA_sb[:], lhsA[:])
            rhsA_sb = inpool.tile([K, M], dt.bfloat16, tag="rhsA")
            nc.sync.dma_start(rhsA_sb[:], rhsA[:])
            lhsB_sb = inpool.tile([K, M], dt.bfloat16, tag="lhsB")
            nc.sync.dma_start(lhsB_sb[:], lhsB[:])
            rhsB_sb = inpool.tile([K, HALF], dt.bfloat16, tag="rhsB")
            nc.sync.dma_start(rhsB_sb[:], rhsB[:])

            minsA_sb = outpool.tile([128, HALF // 128], dt.float32, tag="minsA")
            minsB_sb = outpool.tile([128, M // 128], dt.float32, tag="minsB")

            def orientation(lhs_sb, rhs_sb, mins_sb, nblocks, free_len):
                # free_len columns per output row; consumed in waves of 2048
                # (4 matmuls x 512 = 4 PSUM banks per wave).
                waves = free_len // 2048
                for blk in range(nblocks):
                    lhs_slice = lhs_sb[:, blk * 128 : (blk + 1) * 128]
                    wm = wmpool.tile([128, waves], dt.float32, tag="wm")
                    for w in range(waves):
                        ps = pspool.tile([128, 2048], dt.float32, tag="ps")
                        for j in range(4):
                            t = w * 4 + j
                            nc.tensor.matmul(
                                ps[:, j * 512 : (j + 1) * 512],
                                lhs_slice,
                                rhs_sb[:, t * 512 : (t + 1) * 512],
                                start=True,
                                stop=True,
                            )
                        nc.vector.tensor_reduce(
                            wm[:, w : w + 1],
                            ps[:],
                            axis=mybir.AxisListType.X,
                            op=mybir.AluOpType.min,
                        )
                    nc.vector.tensor_reduce(
                        mins_sb[:, blk : blk + 1],
                        wm[:],
                        axis=mybir.AxisListType.X,
                        op=mybir.AluOpType.min,
                    )

            orientation(lhsA_sb, rhsA_sb, minsA_sb, HALF // 128, M)
            orientation(lhsB_sb, rhsB_sb, minsB_sb, M // 128, HALF)

            nc.sync.dma_start(minsA[:], minsA_sb[:])
            nc.sync.dma_start(minsB[:], minsB_sb[:])

    nc.compile()
    return nc


def _get_program():
    if "nc" not in _PROGRAM_CACHE:
        _PROGRAM_CACHE["nc"] = _build_program()
    return _PROGRAM_CACHE["nc"]


def _get_runner():
    """Cached jitted 8-core executor (mirrors bass2jax.run_bass_via_pjrt's
    multi-core path, but keeps one jax.jit callable alive so repeat kernel()
    calls skip retracing/relowering)."""
    if "runner" in _PROGRAM_CACHE:
        return _PROGRAM_CACHE["runner"]

    import jax
    import jax.numpy as jnp  # noqa: F401
    from jax.sharding import Mesh, PartitionSpec

    try:
        from jax.experimental.shard_map import shard_map
    except ImportError:
        from jax.shard_map import shard_map  # newer jax

    import concourse.mybir as mybir
    from concourse import bass2jax

    nc = _get_program()
    bass2jax.install_neuronx_cc_hook()

    partition_name = (
        nc.partition_id_tensor.name if nc.partition_id_tensor is not None else None
    )
    in_names, out_names, out_avals = [], [], []
    for alloc in nc.m.functions[0].allocations:
        if not isinstance(alloc, mybir.MemoryLocationSet):
            continue
        name = alloc.memorylocations[0].name
        if alloc.kind == "ExternalInput":
            if name != partition_name:
                in_names.append(name)
        elif alloc.kind == "ExternalOutput":
            out_names.append(name)
            out_avals.append(
                jax.core.ShapedArray(
                    tuple(alloc.tensor_shape), mybir.dt.np(alloc.dtype)
                )
            )
    n_params = len(in_names)
    n_outs = len(out_avals)
    all_in_names = list(in_names) + list(out_names)
    if partition_name is not None:
        all_in_names.append(partition_name)

    def _body(*args):
        operands = list(args)
        if partition_name is not None:
            operands.append(bass2jax.partition_id_tensor())
        outs = bass2jax._bass_exec_p.bind(
            *operands,
            out_avals=tuple(out_avals),
            in_names=tuple(all_in_names),
            out_names=tuple(out_names),
            lowering_input_output_aliases=(),
            sim_require_finite=True,
            sim_require_nnan=True,
            nc=nc,
        )
        return tuple(outs)

    devices = jax.devices()[:NCORES]
    assert len(devices) == NCORES, f"need {NCORES} cores, got {len(jax.devices())}"
    mesh = Mesh(np.asarray(devices), ("core",))
    donate = tuple(range(n_params, n_params + n_outs))
    sharded = jax.jit(
        shard_map(
            _body,
            mesh=mesh,
            in_specs=(PartitionSpec("core"),) * (n_params + n_outs),
            out_specs=(PartitionSpec("core"),) * n_outs,
            check_rep=False,
        ),
        donate_argnums=donate,
        keep_unused=True,
    )

    def runner(in_maps):
        concat_in = [
            np.concatenate([np.asarray(m[name]) for m in in_maps], axis=0)
            for name in in_names
        ]
        concat_zeros = [
            np.zeros((NCORES * a.shape[0], *a.shape[1:]), a.dtype) for a in out_avals
        ]
        out_arrs = sharded(*concat_in, *concat_zeros)
        return [
            {
                name: np.asarray(out_arrs[i]).reshape(
                    NCORES, *out_avals[i].shape
                )[c]
                for i, name in enumerate(out_names)
            }
            for c in range(NCORES)
        ]

    _PROGRAM_CACHE["runner"] = runner
    return runner


class _Results:
    """Duck-typed stand-in for BassKernelResults from the cached runner."""

    def __init__(self, results):
        self.results = results
        self.exec_time_ns = None
        self.mean_exec_time_ns = None
        self.max_exec_time_core_id = None
        self.instructions_and_trace = None


def _run_device(in_maps, trace=False):
    if trace:
        from concourse import bass_utils

        try:
            return bass_utils.run_bass_kernel_spmd(
                _get_program(),
                in_maps,
                core_ids=list(range(NCORES)),
                trace=True,
                trace_cores=list(range(NCORES)),
            )
        except (ModuleNotFoundError, ImportError, AttributeError) as e:
            print(f"[kernel] trace unavailable ({e!r}); running without trace")
    return _Results(_get_runner()(in_maps))


def _run(transformed_source, transformed_target, trace=False):
    x = np.ascontiguousarray(np.asarray(transformed_source, dtype=np.float32))
    y = np.ascontiguousarray(np.asarray(transformed_target, dtype=np.float32))
    assert x.shape == (B, N, D) and y.shape == (B, M, D)

    x2 = np.einsum("bnd,bnd->bn", x, x)  # [B, N] fp32
    y2 = np.einsum("bmd,bmd->bm", y, y)                  # [B, M] fp32

    in_maps = []
    for core in range(NCORES):
        b, h = core // 2, core % 2
        xh = x[b, h * HALF : (h + 1) * HALF]
        yf = y[b]
        lA, rA = _build_rows(xh, yf, y2[b])
        lB, rB = _build_rows(yf, xh, x2[b, h * HALF : (h + 1) * HALF])
        in_maps.append({"lhsA": lA, "rhsA": rA, "lhsB": lB, "rhsB": rB})

    res = _run_device(in_maps, trace=trace)

    cham_x_acc = 0.0
    cham_y_acc = 0.0
    for core in range(NCORES):
        b, h = core // 2, core % 2
        rmA = res.results[core]["minsA"].T.reshape(-1).astype(np.float64)
        cham_x_acc += np.sum(rmA + x2[b, h * HALF : (h + 1) * HALF].astype(np.float64))
    for b in range(B):
        m0 = res.results[2 * b]["minsB"].T.reshape(-1)
        m1 = res.results[2 * b + 1]["minsB"].T.reshape(-1)
        cm = np.minimum(m0, m1).astype(np.float64) + y2[b].astype(np.float64)
        cham_y_acc += np.sum(cm)

    cham_x = cham_x_acc / (B * N)
    cham_y = cham_y_acc / (B * M)
    loss = np.asarray(0.01 * (cham_x + cham_y), dtype=np.float32)
    return loss, res


def kernel(transformed_source, transformed_target):
    loss, _ = _run(transformed_source, transformed_target)
    return loss
